# revision 85
# baseline (speedup 1.0000x reference)
"""GCN (GraphConv norm='both' -> ReLU -> SAGEConv mean) on 8 Trainium2 NeuronCores.

Contract: kernel(**inputs) takes the FULL inputs from setup_inputs() and
returns the FULL [N, OUT] output.

Sharding strategy (graph/data parallel, per the problem's sharding hint):
  - Nodes are partitioned contiguously across the 8 cores (12500 each).
  - Edges are partitioned by the owner of their *dst* node; each core's
    edges are bucketed by (128-node dst window, 32768-row src range) --
    the range split because dma_gather indices are int16 -- and padded to
    128-edge chunks. Chunk structure is the max over cores so the SPMD
    program is identical on all 8.
  - Weights are replicated (cast to bf16 host-side; all matmuls run bf16
    at 1 cycle/row instead of fp32's 4).
  - Phase 1 (per core): dma_gather x[src] rows (bf16, 256B rows) from HBM
    with ONE large gather per (8-window group x src range) cell on 4 SWDGE
    queues, segment-sum via one-hot matmuls on the TensorEngine into PSUM
    (the edge weight s_out[src]*s_in[dst] folded into the one-hot values,
    built batched per window on the VectorEngine in bf16), then
    hT = relu(W1.T @ aggT + b1) kept SBUF-resident, and z = h @ W_neigh
    written to a local z shard.
  - z shards are AllGathered across the 8 cores (the halo exchange -- on
    this random graph the halo is ~the whole graph, and exchanging
    z = h @ W_neigh (64 wide) instead of h (128 wide) halves the traffic
    since (segsum h) @ W_neigh == segsum (h @ W_neigh)).
  - Phase 2 (per core): dma_gather z[src] rows (f32, 256B), convert slabs
    to bf16, segment-sum with 0/1 one-hots, scale by 1/deg_in per dst row
    (partition-wise), add h @ W_self + b2, write the core's [12500, 64]
    output shard.
  - Host concatenates the 8 shards.

Host-side prep is integer graph restructuring (edge partition / sort /
pad / degree counts), dtype casts, and the per-edge phase-1 normalization
weights derived from the degree histograms; all O(N*F) / O(E*F) floating
point work (gathers, segment sums, matmuls, bias, relu, 1/deg scaling)
runs on the NeuronCores.

Pipelining: the per-window "tail" ops (W1 matmul, relu, z matmul / the
SAGE combine) of group g-1 are emitted between group g's gathers and
chunk matmuls so the TensorEngine never stalls on Scalar-engine results
and stays at its ramped p-state.
"""

import os
import sys
from contextlib import ExitStack

import numpy as np

for _p in ("/opt/trn_rl_repo", "/opt/pypackages"):
    if _p not in sys.path:
        sys.path.append(_p)

import ml_dtypes

import concourse.bacc as bacc
import concourse.bass as bass
import concourse.mybir as mybir
import concourse.tile as tile
from concourse.bass_utils import run_bass_kernel_spmd

F32 = mybir.dt.float32
BF16 = mybir.dt.bfloat16
FP8 = mybir.dt.float8e4
I16 = mybir.dt.int16
AOT = mybir.AluOpType
AFT = mybir.ActivationFunctionType
BF = ml_dtypes.bfloat16

N_CORES = 8
WIN = 128
MAXRANGE = 32768  # dma_gather idx is int16
GROUP1 = 6  # windows per gather slab group
SUBCHUNKS = 8  # max chunks per dma_gather instruction (1024-idx ucode limit)
NQUEUES = 4


def _install_ntff_hook_shim():
    """The agent image's antenv lacks axon_hooks; provide it so trace=True
    can capture NTFF profiles through libaxon (same hook trn_boot would
    register). No-op if the real module exists or libaxon lacks support."""
    try:
        from antenv import axon_hooks  # noqa: F401
        return
    except ImportError:
        pass
    try:
        import types

        import antenv
        from trn_agent_boot.trn_boot import _ntff_profile_via_ctypes

        mod = types.ModuleType("antenv.axon_hooks")
        mod._hook = _ntff_profile_via_ctypes("/opt/axon/libaxon_pjrt.so")

        def get_axon_ntff_profile_hook():
            return mod._hook

        def set_axon_ntff_profile_hook(h):
            mod._hook = h

        mod.get_axon_ntff_profile_hook = get_axon_ntff_profile_hook
        mod.set_axon_ntff_profile_hook = set_axon_ntff_profile_hook
        sys.modules["antenv.axon_hooks"] = mod
        antenv.axon_hooks = mod
    except Exception:
        pass


_install_ntff_hook_shim()


# ---------------------------------------------------------------------------
# Host-side graph prep
# ---------------------------------------------------------------------------

class Prep:
    pass


def prepare(src, dst, n_nodes, n_cores=N_CORES):
    src = np.asarray(src).astype(np.int64)
    dst = np.asarray(dst).astype(np.int64)
    P = n_nodes // n_cores
    assert P * n_cores == n_nodes
    NW = (P + WIN - 1) // WIN
    rows_last = P - WIN * (NW - 1)
    RSZ = MAXRANGE if n_nodes > MAXRANGE else -(-n_nodes // 4)
    NRANGES = -(-n_nodes // RSZ)
    assert RSZ <= MAXRANGE

    deg_out = np.bincount(src, minlength=n_nodes).astype(np.float32)
    deg_in = np.bincount(dst, minlength=n_nodes).astype(np.float32)
    s_out = 1.0 / np.sqrt(np.maximum(deg_out, 1.0))
    s_in = 1.0 / np.sqrt(np.maximum(deg_in, 1.0))
    sw1_all = (s_out[src] * s_in[dst]).astype(np.float32)

    # per-dst-node 1/max(deg_in,1) in [partition, window] layout per core
    invd = (1.0 / np.maximum(deg_in, 1.0)).astype(np.float32)

    owner = dst // P
    ldst = dst - owner * P
    wrow = ldst // WIN
    code = (ldst % WIN).astype(np.float32)
    rng_of = src // RSZ

    counts = np.zeros((n_cores, NW, NRANGES), np.int64)
    np.add.at(counts, (owner, wrow, rng_of), 1)
    cwr = (counts.max(axis=0) + WIN - 1) // WIN  # [NW, NRANGES]
    empty = cwr.sum(axis=1) == 0
    cwr[empty, 0] = 1
    n_w = cwr.sum(axis=1)  # chunks per window

    groups1 = [(g0, min(g0 + GROUP1, NW)) for g0 in range(0, NW, GROUP1)]

    def layout(cwr_, nr):
        """Gather-order (group -> range -> window -> chunk) layout."""
        n_w_ = cwr_.sum(axis=1)
        cell_start = np.zeros((NW, nr), np.int64)
        slab_meta = []
        c = 0
        for g0, g1 in groups1:
            metas = []
            for r in range(nr):
                s = c
                for w in range(g0, g1):
                    cell_start[w, r] = c
                    c += int(cwr_[w, r])
                metas.append((s, c - s))
            slab_meta.append(metas)
        C_ = c
        gathers = []
        for g in range(len(groups1)):
            for r in range(nr):
                s, n = slab_meta[g][r]
                if n == 0:
                    continue
                # balanced split into ceil(n/SUBCHUNKS) near-equal gathers
                parts = -(-n // SUBCHUNKS)
                bounds_ = [n * j // parts for j in range(parts + 1)]
                for j in range(parts):
                    i, nb = bounds_[j], bounds_[j + 1] - bounds_[j]
                    gathers.append((g, r, i, nb, s + i))
        wc0_ = np.zeros(NW, np.int64)
        wc0_[1:] = np.cumsum(n_w_)[:-1]
        window_chunks = []
        for w in range(NW):
            lst = []
            for r in range(nr):
                for j in range(int(cwr_[w, r])):
                    lst.append((r, int(cell_start[w, r]) + j))
            window_chunks.append(lst)
        return cell_start, slab_meta, gathers, wc0_, window_chunks, C_, n_w_

    cell_start, slab_meta, gathers, wc0, window_chunks, C, n_w = layout(
        cwr, NRANGES
    )

    # phase-2 gathers fetch bf16 (z, z-next) pair rows (256B) from two
    # half-shard AllGather outputs; the first half's collective fires while
    # phase 1 is still running so only the second half gates phase 2
    HWIN = (NW + 1) // 2
    b_starts = [0, min(HWIN * WIN, P), P]
    b_sizes = [b_starts[1] - b_starts[0], b_starts[2] - b_starts[1]]
    assert all(b % 2 == 0 for b in b_sizes) and all(
        4 * b <= 32768 for b in b_sizes
    )
    NRANGES2 = 2
    s_owner = src // P
    s_local = src - s_owner * P
    rng2_of = (s_local >= b_starts[1]).astype(np.int64)
    idx2_of = (
        s_owner * np.array(b_sizes)[rng2_of]
        + s_local
        - np.array(b_starts[:-1])[rng2_of]
    ) // 2
    counts2 = np.zeros((n_cores, NW, NRANGES2), np.int64)
    np.add.at(counts2, (owner, wrow, rng2_of), 1)
    cwr2 = (counts2.max(axis=0) + WIN - 1) // WIN
    empty2 = cwr2.sum(axis=1) == 0
    cwr2[empty2, 0] = 1
    cell_start2, slab_meta2, gathers2, wc02, window_chunks2, C2, n_w2 = layout(
        cwr2, NRANGES2
    )

    def wrap16(a_idx, c_):
        # 16-partition engine wrap, replicated to 128 partitions, per the
        # dma_gather idx layout; one column block of 8 per chunk.
        e = np.ascontiguousarray(np.tile(a_idx.reshape(-1, 16).T, (8, 1)))
        assert e.shape == (128, c_ * 8)
        return e

    per_core = []
    for k in range(n_cores):
        m = owner == k
        e_src = src[m]
        e_code_all = code[m]
        e_sw1_all = sw1_all[m]
        e_rng = rng_of[m]
        e_rng2 = rng2_of[m]
        e_wrow = wrow[m]

        # ---- phase 1 layout (sorted by window, src range) ----
        key = e_wrow * NRANGES + e_rng
        order = np.argsort(key, kind="stable")
        s_src = e_src[order]
        s_key = key[order]
        s_code = e_code_all[order]
        s_sw1 = e_sw1_all[order]
        bounds = np.searchsorted(s_key, np.arange(NW * NRANGES + 1))

        # gather-order idx array (pads = 0: they gather row 0 harmlessly and
        # their zero one-hot rows contribute nothing); window-major arrays
        A_idx = np.full(C * WIN, 0, np.int16)
        W_code = np.full(C * WIN, 255.0, np.float32)
        W_sw1 = np.zeros(C * WIN, np.float32)
        for w in range(NW):
            woff = 0
            for r in range(NRANGES):
                a, b = bounds[w * NRANGES + r], bounds[w * NRANGES + r + 1]
                n = b - a
                gbase = int(cell_start[w, r]) * WIN
                wbase = (int(wc0[w]) + woff) * WIN
                woff += int(cwr[w, r])
                if n == 0:
                    continue
                A_idx[gbase : gbase + n] = (s_src[a:b] - r * RSZ).astype(np.int16)
                W_code[wbase : wbase + n] = s_code[a:b]
                W_sw1[wbase : wbase + n] = s_sw1[a:b]

        # ---- phase 2 layout (sorted by window, pair range; parity-split
        # one-hot codes select the z[2i] / z[2i+1] half of each pair row) ----
        e_idx2 = idx2_of[m]
        e_par = (src[m] & 1).astype(np.int64)
        key2 = e_wrow * NRANGES2 + e_rng2
        order2 = np.argsort(key2, kind="stable")
        p_idx = e_idx2[order2]
        p_par = e_par[order2]
        p_key = key2[order2]
        p_code = e_code_all[order2]
        bounds2 = np.searchsorted(p_key, np.arange(NW * NRANGES2 + 1))

        A_idx2 = np.full(C2 * WIN, 0, np.int16)
        W_ev = np.full(C2 * WIN, 255.0, np.float32)
        W_od = np.full(C2 * WIN, 255.0, np.float32)
        for w in range(NW):
            woff = 0
            for r in range(NRANGES2):
                a, b = bounds2[w * NRANGES2 + r], bounds2[w * NRANGES2 + r + 1]
                n = b - a
                gbase = int(cell_start2[w, r]) * WIN
                wbase = (int(wc02[w]) + woff) * WIN
                woff += int(cwr2[w, r])
                if n == 0:
                    continue
                A_idx2[gbase : gbase + n] = p_idx[a:b].astype(np.int16)
                ev = p_par[a:b] == 0
                cseg = p_code[a:b]
                W_ev[wbase : wbase + n] = np.where(ev, cseg, 255.0)
                W_od[wbase : wbase + n] = np.where(ev, 255.0, cseg)

        def tr(a, c_, dt):
            return np.ascontiguousarray(a.reshape(c_, WIN).T.astype(dt))

        def onehot8(codes_flat, c_):
            # host-expanded 0/1 one-hot (exact in fp8), [WIN, c_*WIN]
            codes = codes_flat.reshape(c_, WIN).T  # [WIN, c_]
            oh = codes[:, :, None] == np.arange(WIN, dtype=np.float32)
            return np.ascontiguousarray(
                oh.astype(ml_dtypes.float8_e4m3).reshape(WIN, c_ * WIN)
            )

        # invd in [partition, window] layout for this core's nodes
        nodes = np.arange(P) + k * P
        iv = np.zeros(NW * WIN, np.float32)
        iv[:P] = invd[nodes]
        invd_pw = np.ascontiguousarray(iv.reshape(NW, WIN).T)

        per_core.append(
            dict(eidx=wrap16(A_idx, C), ecode=tr(W_code, C, BF),
                 esw1=tr(W_sw1, C, BF), eidx2=wrap16(A_idx2, C2),
                 eqev=onehot8(W_ev, C2), eqod=onehot8(W_od, C2),
                 invd=invd_pw)
        )

    p = Prep()
    p.P, p.NW, p.rows_last, p.C, p.RSZ = P, NW, rows_last, C, RSZ
    p.nranges = NRANGES
    p.cwr = cwr
    p.n_w = n_w
    p.wc0 = wc0
    p.groups1 = groups1
    p.slab_meta = slab_meta
    p.gathers = gathers
    p.NG = len(gathers)
    p.window_chunks = window_chunks
    p.C2 = C2
    p.nranges2 = NRANGES2
    p.b_sizes = b_sizes
    p.b_starts = b_starts
    p.fire0 = -(-HWIN // GROUP1)  # groups after which half 0 is complete
    p.cwr2 = cwr2
    p.n_w2 = n_w2
    p.wc02 = wc02
    p.slab_meta2 = slab_meta2
    p.gathers2 = gathers2
    p.window_chunks2 = window_chunks2
    p.per_core = per_core
    p.n_nodes = n_nodes
    p.n_cores = n_cores
    return p


# ---------------------------------------------------------------------------
# Bass/Tile kernel builder
# ---------------------------------------------------------------------------

def build_gcn(p, F, H, O, gather_bufs=3, gather_bufs2=3):
    NW, C, P, RSZ = p.NW, p.C, p.P, p.RSZ
    NRANGES = p.nranges
    C2, NRANGES2 = p.C2, p.nranges2
    max_nw = int(p.n_w.max())
    max_nw2 = int(p.n_w2.max())
    max_slab = [
        max((p.slab_meta[g][r][1] for g in range(len(p.groups1))), default=0)
        for r in range(NRANGES)
    ]
    max_slab2 = [
        max((p.slab_meta2[g][r][1] for g in range(len(p.groups1))), default=0)
        for r in range(NRANGES2)
    ]
    # gathers grouped by (g, r): list of (chunk_off_in_slab, nb, chunk_start)
    by_slab = {}
    for g, r, i, nb, cs in p.gathers:
        by_slab.setdefault((g, r), []).append((i, nb, cs))
    by_slab2 = {}
    for g, r, i, nb, cs in p.gathers2:
        by_slab2.setdefault((g, r), []).append((i, nb, cs))
    # max window-major chunk columns per group (for the fp8 one-hot tiles)
    max_geq = max(
        int(p.wc02[g1 - 1] + p.n_w2[g1 - 1] - p.wc02[g0])
        for g0, g1 in p.groups1
    )

    nc = bacc.Bacc(
        "TRN2", debug=False, enable_asserts=False, num_devices=p.n_cores,
        num_swdge_queues=NQUEUES,
    )

    x_d = nc.dram_tensor("x", [p.n_nodes, F], BF16, kind="ExternalInput").ap()
    W1_d = nc.dram_tensor("W1", [F, H], BF16, kind="ExternalInput").ap()
    b1_d = nc.dram_tensor("b1", [H, 1], F32, kind="ExternalInput").ap()
    Ws_d = nc.dram_tensor("W_self", [H, O], BF16, kind="ExternalInput").ap()
    Wn_d = nc.dram_tensor("W_neigh", [H, O], BF16, kind="ExternalInput").ap()
    b2_d = nc.dram_tensor("b2", [1, O], BF16, kind="ExternalInput").ap()
    eidx_d = nc.dram_tensor("eidx", [WIN, C * 8], I16, kind="ExternalInput").ap()
    ecode_d = nc.dram_tensor("ecode", [WIN, C], BF16, kind="ExternalInput").ap()
    esw1_d = nc.dram_tensor("esw1", [WIN, C], BF16, kind="ExternalInput").ap()
    eidx2_d = nc.dram_tensor(
        "eidx2", [WIN, C2 * 8], I16, kind="ExternalInput"
    ).ap()
    eqev_d = nc.dram_tensor(
        "eqev", [WIN, C2 * WIN], FP8, kind="ExternalInput"
    ).ap()
    eqod_d = nc.dram_tensor(
        "eqod", [WIN, C2 * WIN], FP8, kind="ExternalInput"
    ).ap()
    invd_d = nc.dram_tensor("invd", [WIN, NW], F32, kind="ExternalInput").ap()
    out_d = nc.dram_tensor("out", [P, O], F32, kind="ExternalOutput").ap()

    qn = [0]

    def next_q():
        q = qn[0]
        qn[0] = (q + 1) % NQUEUES
        return q

    with tile.TileContext(nc, num_cores=p.n_cores) as tc, ExitStack() as ctx:
        const = ctx.enter_context(tc.tile_pool(name="const", bufs=1))
        dram = ctx.enter_context(tc.tile_pool(name="dram", bufs=1, space="DRAM"))

        W1s = const.tile([F, H], BF16)
        nc.sync.dma_start(W1s[:], W1_d)
        Wss = const.tile([H, O], BF16)
        nc.sync.dma_start(Wss[:], Ws_d)
        Wns = const.tile([H, O], BF16)
        nc.sync.dma_start(Wns[:], Wn_d)
        b1s = const.tile([H, 1], F32)
        nc.sync.dma_start(b1s[:], b1_d)
        b2s = const.tile([1, O], BF16)
        nc.sync.dma_start(b2s[:], b2_d)
        invd_s = const.tile([WIN, NW], F32)
        nc.sync.dma_start(invd_s[:], invd_d)

        ones1 = const.tile([1, WIN], BF16)
        nc.vector.memset(ones1[:], 1.0)
        iota = const.tile([WIN, WIN], BF16)
        nc.gpsimd.iota(
            iota[:],
            pattern=[[1, WIN]],
            base=0,
            channel_multiplier=0,
            allow_small_or_imprecise_dtypes=True,
        )

        hT = const.tile([H, NW * WIN], BF16)

        zshard = dram.tile([P, O], BF16)
        # halo-exchanged z halves, viewed as bf16 pair rows so the phase-2
        # gather descriptors are 256B like phase 1's
        zh = [
            dram.tile([4 * p.b_sizes[b], 2 * O], BF16, addr_space="Shared",
                      name=f"zh{b}", tag=f"zh{b}")
            for b in range(2)
        ]
        # self-term b2 + h @ W_self for every window, filled during the
        # halo exchange so the PE isn't idle while the collective runs
        sb_all = const.tile([WIN, NW * O], BF16)

        def gather_slab(pool, g, r, src_ap, elem, dt, tag, idxs, memset):
            s, n = p.slab_meta[g][r]
            if n == 0:
                return None, s
            t = pool.tile([WIN, max_slab[r], elem], dt, tag=tag)
            if memset:
                nc.vector.memset(t[:], 0.0)
            r0 = r * RSZ
            r1 = min(r0 + RSZ, p.n_nodes)
            for i, nb, cs in by_slab[(g, r)]:
                nc.gpsimd.dma_gather(
                    out_ap=t[:, i : i + nb, :],
                    in_ap=src_ap[r0:r1, :],
                    idxs_ap=idxs[:, cs * 8 : (cs + nb) * 8],
                    num_idxs=nb * WIN,
                    num_idxs_reg=nb * WIN,
                    elem_size=elem,
                    queue_num=next_q(),
                )
            return t, s

        def build_eq(pool, codes, n, c0, mx, tag, weighted=False):
            """Batched one-hot over n chunk columns of `codes`."""
            eq = pool.tile([WIN, mx, WIN], BF16, tag=tag)
            nc.vector.tensor_tensor(
                out=eq[:, :n, :],
                in0=codes[:, c0 : c0 + n].to_broadcast([WIN, n, WIN]),
                in1=iota[:].rearrange("p f -> p () f").to_broadcast([WIN, n, WIN]),
                op=AOT.is_equal,
            )
            if weighted:
                nc.vector.tensor_tensor(
                    out=eq[:, :n, :],
                    in0=eq[:, :n, :],
                    in1=esw1_s[:, c0 : c0 + n].to_broadcast([WIN, n, WIN]),
                    op=AOT.mult,
                )
            return eq

        # ---------------- phase 1 ----------------
        groups1 = p.groups1
        with (
            tc.tile_pool(name="idx1", bufs=1) as idx1p,
            tc.tile_pool(name="xg", bufs=gather_bufs) as xgp,
            tc.tile_pool(name="oh1", bufs=2) as ohp,
            tc.tile_pool(name="aggn", bufs=2 * GROUP1 + 2) as aggp,
            tc.tile_pool(name="zt", bufs=2) as ztp,
            tc.tile_pool(name="psA", bufs=3, space="PSUM") as psA,
            tc.tile_pool(name="psH", bufs=2, space="PSUM") as psH,
            tc.tile_pool(name="psZ", bufs=2, space="PSUM") as psZ,
        ):
            eidx_s = idx1p.tile([WIN, C * 8], I16)
            nc.sync.dma_start(eidx_s[:], eidx_d)
            ecode_s = idx1p.tile([WIN, C], BF16)
            nc.sync.dma_start(ecode_s[:], ecode_d)
            esw1_s = idx1p.tile([WIN, C], BF16)
            nc.sync.dma_start(esw1_s[:], esw1_d)

            pend1 = []  # (w, rows, aggn tile) awaiting tail ops
            fired = [False, False]

            def fire_half(done_groups):
                for b in range(2):
                    if fired[b]:
                        continue
                    need = p.fire0 if b == 0 else len(groups1)
                    if done_groups >= need:
                        bs, bn = p.b_starts[b], p.b_sizes[b]
                        nc.gpsimd.collective_compute(
                            "AllGather",
                            AOT.bypass,
                            replica_groups=[list(range(p.n_cores))],
                            ins=[zshard.opt()[bs : bs + bn, :]],
                            outs=[zh[b].opt()],
                        )
                        fired[b] = True

            def tail1():
                for w, rows, aggn in pend1:
                    wsl = slice(w * WIN, (w + 1) * WIN)
                    hpre = psH.tile([H, WIN], F32, tag="hpre")
                    nc.tensor.matmul(
                        out=hpre[:], lhsT=W1s[:], rhs=aggn[:], start=True,
                        stop=True,
                    )
                    nc.scalar.activation(hT[:, wsl], hpre[:], AFT.Relu, bias=b1s[:])
                for w, rows, aggn in pend1:
                    wsl = slice(w * WIN, (w + 1) * WIN)
                    zp = psZ.tile([WIN, O], F32, tag="zp")
                    nc.tensor.matmul(
                        out=zp[:], lhsT=hT[:, wsl], rhs=Wns[:], start=True,
                        stop=True,
                    )
                    zt = ztp.tile([WIN, O], BF16, tag="zt")
                    nc.vector.tensor_copy(zt[:], zp[:])
                    nc.sync.dma_start(
                        zshard[w * WIN : w * WIN + rows, :], zt[:rows, :]
                    )
                pend1.clear()

            for g, (g0, g1) in enumerate(groups1):
                slabs = {}
                for r in range(NRANGES):
                    t, s = gather_slab(
                        xgp, g, r, x_d, F, BF16, f"xg{r}", eidx_s,
                        memset=g < gather_bufs,
                    )
                    if t is not None:
                        slabs[r] = (t, s)

                tail1()  # tails of group g-1 overlap group g's gathers
                fire_half(g)

                for w in range(g0, g1):
                    rows = p.rows_last if w == NW - 1 else WIN
                    chunks = p.window_chunks[w]

                    eq = build_eq(
                        ohp, ecode_s, int(p.n_w[w]), int(p.wc0[w]), max_nw,
                        "eq", weighted=True,
                    )
                    agg = psA.tile([F, WIN], F32, tag="agg")
                    for jj, (r, gid) in enumerate(chunks):
                        t, s = slabs[r]
                        nc.tensor.matmul(
                            out=agg[:],
                            lhsT=t[:, gid - s, :],
                            rhs=eq[:, jj, :],
                            start=(jj == 0),
                            stop=(jj == len(chunks) - 1),
                        )

                    aggn = aggp.tile([F, WIN], BF16, tag="aggn")
                    nc.scalar.activation(aggn[:], agg[:], AFT.Copy)
                    pend1.append((w, rows, aggn))

            tail1()
            fire_half(len(groups1))

        # overlap the collective with the self-term matmuls (independent of z)
        with tc.tile_pool(name="psB", bufs=3, space="PSUM") as psB:
            for w in range(NW):
                wsl = slice(w * WIN, (w + 1) * WIN)
                sb = psB.tile([WIN, O], F32, tag="sb")
                nc.tensor.matmul(
                    out=sb[:], lhsT=ones1[:], rhs=b2s[:], start=True, stop=False
                )
                nc.tensor.matmul(
                    out=sb[:], lhsT=hT[:, wsl], rhs=Wss[:], start=False, stop=True
                )
                nc.scalar.activation(sb_all[:, w * O : (w + 1) * O], sb[:], AFT.Copy)

        # ---------------- phase 2 ----------------
        with (
            tc.tile_pool(name="idx2", bufs=1) as idx2p,
            tc.tile_pool(name="zg", bufs=gather_bufs2) as zgp,
            tc.tile_pool(name="oh2", bufs=2) as ohp2,
            tc.tile_pool(name="nm", bufs=2) as nmp,
            tc.tile_pool(name="nmc", bufs=2 * GROUP1 + 2) as nmcp,
            tc.tile_pool(name="ot", bufs=2) as otp,
            tc.tile_pool(name="psN", bufs=3, space="PSUM") as psN,
        ):
            eidx2_s = idx2p.tile([WIN, C2 * 8], I16)
            nc.sync.dma_start(eidx2_s[:], eidx2_d)

            pend2 = []  # (w, rows, nm SBUF copy)

            def tail2():
                for w, rows, nm in pend2:
                    # nm * invd[dst] (partition-wise) then + (h@Ws + b2)
                    nms = nmp.tile([WIN, O], F32, tag="nms")
                    nc.vector.tensor_scalar(
                        out=nms[:], in0=nm[:], scalar1=invd_s[:, w : w + 1],
                        scalar2=None, op0=AOT.mult,
                    )
                    outt = otp.tile([WIN, O], F32, tag="outt")
                    nc.vector.tensor_tensor(
                        outt[:], nms[:], sb_all[:, w * O : (w + 1) * O],
                        op=AOT.add,
                    )
                    nc.sync.dma_start(
                        out_d[w * WIN : w * WIN + rows, :], outt[:rows, :]
                    )
                pend2.clear()

            for g, (g0, g1) in enumerate(groups1):
                slabs = {}
                for r in range(NRANGES2):
                    s, n = p.slab_meta2[g][r]
                    if n == 0:
                        continue
                    t = zgp.tile([WIN, max_slab2[r], 2 * O], BF16, tag=f"zg{r}")
                    if g < gather_bufs2:
                        nc.vector.memset(t[:], 0.0)
                    for i, nb, cs in by_slab2[(g, r)]:
                        nc.gpsimd.dma_gather(
                            out_ap=t[:, i : i + nb, :],
                            in_ap=zh[r].opt(),
                            idxs_ap=eidx2_s[:, cs * 8 : (cs + nb) * 8],
                            num_idxs=nb * WIN,
                            num_idxs_reg=nb * WIN,
                            elem_size=2 * O,
                            queue_num=next_q(),
                        )
                    slabs[r] = (t, s)

                # host-precomputed fp8 parity one-hots for this group's
                # window-major chunk columns (replaces DVE is_equal builds)
                q0 = int(p.wc02[g0])
                q1 = int(p.wc02[g1 - 1] + p.n_w2[g1 - 1])
                eqe = ohp2.tile([WIN, max_geq, WIN], FP8, tag="eqe")
                nc.sync.dma_start(
                    eqe[:, 0 : q1 - q0, :], eqev_d[:, q0 * WIN : q1 * WIN]
                )
                eqo = ohp2.tile([WIN, max_geq, WIN], FP8, tag="eqo")
                nc.sync.dma_start(
                    eqo[:, 0 : q1 - q0, :], eqod_d[:, q0 * WIN : q1 * WIN]
                )

                tail2()

                for w in range(g0, g1):
                    rows = p.rows_last if w == NW - 1 else WIN
                    chunks = p.window_chunks2[w]
                    c20 = int(p.wc02[w])

                    nm = psN.tile([WIN, O], F32, tag="nm")
                    for jj, (r, gid) in enumerate(chunks):
                        t, s = slabs[r]
                        jq = c20 - q0 + jj
                        nc.tensor.matmul(
                            out=nm[:],
                            lhsT=eqe[:, jq, :],
                            rhs=t[:, gid - s, 0:O],
                            start=(jj == 0),
                            stop=False,
                        )
                        nc.tensor.matmul(
                            out=nm[:],
                            lhsT=eqo[:, jq, :],
                            rhs=t[:, gid - s, O : 2 * O],
                            start=False,
                            stop=(jj == len(chunks) - 1),
                        )
                    nmc = nmcp.tile([WIN, O], F32, tag="nmc")
                    nc.scalar.activation(nmc[:], nm[:], AFT.Copy)
                    pend2.append((w, rows, nmc))

            tail2()

    nc.compile()
    return nc


# ---------------------------------------------------------------------------
# Entry point
# ---------------------------------------------------------------------------

_CACHE = {}


def _get_compiled(p, F, H, O):
    key = (p.n_nodes, p.n_cores, p.C, tuple(map(tuple, p.cwr.tolist())), F, H, O)
    if key not in _CACHE:
        import time as _time

        t0 = _time.time()
        _CACHE[key] = build_gcn(p, F, H, O)
        if os.environ.get("GCN_VERBOSE"):
            print(f"[gcn] build+bass-compile: {_time.time() - t0:.1f}s", flush=True)
    return _CACHE[key]


def make_in_maps(p, inputs):
    H = np.asarray(inputs["W1"]).shape[1]
    O = np.asarray(inputs["W_self"]).shape[1]
    base = dict(
        x=np.ascontiguousarray(np.asarray(inputs["x"], np.float32).astype(BF)),
        W1=np.ascontiguousarray(np.asarray(inputs["W1"], np.float32).astype(BF)),
        b1=np.ascontiguousarray(np.asarray(inputs["b1"], np.float32).reshape(H, 1)),
        W_self=np.ascontiguousarray(
            np.asarray(inputs["W_self"], np.float32).astype(BF)
        ),
        W_neigh=np.ascontiguousarray(
            np.asarray(inputs["W_neigh"], np.float32).astype(BF)
        ),
        b2=np.ascontiguousarray(
            np.asarray(inputs["b2"], np.float32).reshape(1, O).astype(BF)
        ),
    )
    in_maps = []
    for k in range(p.n_cores):
        m = dict(base)
        m.update(p.per_core[k])
        in_maps.append(m)
    return in_maps


def kernel(**inputs):
    x = np.asarray(inputs["x"])
    src = np.asarray(inputs["src"])
    dst = np.asarray(inputs["dst"])
    n_nodes, F = x.shape
    H = np.asarray(inputs["W1"]).shape[1]
    O = np.asarray(inputs["W_self"]).shape[1]

    p = prepare(src, dst, n_nodes)
    nc = _get_compiled(p, F, H, O)
    in_maps = make_in_maps(p, inputs)
    res = run_bass_kernel_spmd(
        nc, in_maps, core_ids=list(range(p.n_cores)),
        trace=bool(int(os.environ.get("GCN_TRACE", "0"))),
    )
    if os.environ.get("GCN_RESULT_HOOK"):
        _CACHE["last_results"] = res
    out = np.concatenate([r["out"] for r in res.results], axis=0)
    return out.astype(np.float32)


# revision 86
# speedup vs baseline: 1.2913x; 1.2913x over previous
"""GCN (GraphConv norm='both' -> ReLU -> SAGEConv mean) on 8 Trainium2 NeuronCores.

Contract: kernel(**inputs) takes the FULL inputs from setup_inputs() and
returns the FULL [N, OUT] output.

Sharding strategy (graph/data parallel, per the problem's sharding hint):
  - Nodes are partitioned contiguously across the 8 cores (12500 each).
  - Edges are partitioned by the owner of their *dst* node; each core's
    edges are bucketed by (128-node dst window, 32768-row src range) --
    the range split because dma_gather indices are int16 -- and padded to
    128-edge chunks. Chunk structure is the max over cores so the SPMD
    program is identical on all 8.
  - Weights are replicated (cast to bf16 host-side; all matmuls run bf16
    at 1 cycle/row instead of fp32's 4).
  - Phase 1 (per core): dma_gather x[src] rows (bf16, 256B rows) from HBM
    with ONE large gather per (8-window group x src range) cell on 4 SWDGE
    queues, segment-sum via one-hot matmuls on the TensorEngine into PSUM
    (the edge weight s_out[src]*s_in[dst] folded into the one-hot values,
    built batched per window on the VectorEngine in bf16), then
    hT = relu(W1.T @ aggT + b1) kept SBUF-resident, and z = h @ W_neigh
    written to a local z shard.
  - z shards are AllGathered across the 8 cores (the halo exchange -- on
    this random graph the halo is ~the whole graph, and exchanging
    z = h @ W_neigh (64 wide) instead of h (128 wide) halves the traffic
    since (segsum h) @ W_neigh == segsum (h @ W_neigh)).
  - Phase 2 (per core): dma_gather z[src] rows (f32, 256B), convert slabs
    to bf16, segment-sum with 0/1 one-hots, scale by 1/deg_in per dst row
    (partition-wise), add h @ W_self + b2, write the core's [12500, 64]
    output shard.
  - Host concatenates the 8 shards.

Host-side prep is integer graph restructuring (edge partition / sort /
pad / degree counts), dtype casts, and the per-edge phase-1 normalization
weights derived from the degree histograms; all O(N*F) / O(E*F) floating
point work (gathers, segment sums, matmuls, bias, relu, 1/deg scaling)
runs on the NeuronCores.

Pipelining: the per-window "tail" ops (W1 matmul, relu, z matmul / the
SAGE combine) of group g-1 are emitted between group g's gathers and
chunk matmuls so the TensorEngine never stalls on Scalar-engine results
and stays at its ramped p-state.
"""

import os
import sys
from contextlib import ExitStack

import numpy as np

for _p in ("/opt/trn_rl_repo", "/opt/pypackages"):
    if _p not in sys.path:
        sys.path.append(_p)

import ml_dtypes

import concourse.bacc as bacc
import concourse.bass as bass
import concourse.mybir as mybir
import concourse.tile as tile
from concourse.bass_utils import run_bass_kernel_spmd

F32 = mybir.dt.float32
BF16 = mybir.dt.bfloat16
FP8 = mybir.dt.float8e4
I16 = mybir.dt.int16
AOT = mybir.AluOpType
AFT = mybir.ActivationFunctionType
BF = ml_dtypes.bfloat16

N_CORES = 8
WIN = 128
MAXRANGE = 32768  # dma_gather idx is int16
GROUP1 = 6  # windows per gather slab group
SUBCHUNKS = 8  # max chunks per dma_gather instruction (1024-idx ucode limit)
NQUEUES = 4


def _install_ntff_hook_shim():
    """The agent image's antenv lacks axon_hooks; provide it so trace=True
    can capture NTFF profiles through libaxon (same hook trn_boot would
    register). No-op if the real module exists or libaxon lacks support."""
    try:
        from antenv import axon_hooks  # noqa: F401
        return
    except ImportError:
        pass
    try:
        import types

        import antenv
        from trn_agent_boot.trn_boot import _ntff_profile_via_ctypes

        mod = types.ModuleType("antenv.axon_hooks")
        mod._hook = _ntff_profile_via_ctypes("/opt/axon/libaxon_pjrt.so")

        def get_axon_ntff_profile_hook():
            return mod._hook

        def set_axon_ntff_profile_hook(h):
            mod._hook = h

        mod.get_axon_ntff_profile_hook = get_axon_ntff_profile_hook
        mod.set_axon_ntff_profile_hook = set_axon_ntff_profile_hook
        sys.modules["antenv.axon_hooks"] = mod
        antenv.axon_hooks = mod
    except Exception:
        pass


_install_ntff_hook_shim()


# ---------------------------------------------------------------------------
# Host-side graph prep
# ---------------------------------------------------------------------------

class Prep:
    pass


def prepare(src, dst, n_nodes, n_cores=N_CORES):
    src = np.asarray(src).astype(np.int64)
    dst = np.asarray(dst).astype(np.int64)
    P = n_nodes // n_cores
    assert P * n_cores == n_nodes
    NW = (P + WIN - 1) // WIN
    rows_last = P - WIN * (NW - 1)
    RSZ = MAXRANGE if n_nodes > MAXRANGE else -(-n_nodes // 4)
    NRANGES = -(-n_nodes // RSZ)
    assert RSZ <= MAXRANGE

    deg_out = np.bincount(src, minlength=n_nodes).astype(np.float32)
    deg_in = np.bincount(dst, minlength=n_nodes).astype(np.float32)
    s_out = 1.0 / np.sqrt(np.maximum(deg_out, 1.0))
    s_in = 1.0 / np.sqrt(np.maximum(deg_in, 1.0))
    sw1_all = (s_out[src] * s_in[dst]).astype(np.float32)

    # per-dst-node 1/max(deg_in,1) in [partition, window] layout per core
    invd = (1.0 / np.maximum(deg_in, 1.0)).astype(np.float32)

    owner = dst // P
    ldst = dst - owner * P
    wrow = ldst // WIN
    code = (ldst % WIN).astype(np.float32)
    rng_of = src // RSZ

    counts = np.zeros((n_cores, NW, NRANGES), np.int64)
    np.add.at(counts, (owner, wrow, rng_of), 1)
    cwr = (counts.max(axis=0) + WIN - 1) // WIN  # [NW, NRANGES]
    empty = cwr.sum(axis=1) == 0
    cwr[empty, 0] = 1
    n_w = cwr.sum(axis=1)  # chunks per window

    groups1 = [(g0, min(g0 + GROUP1, NW)) for g0 in range(0, NW, GROUP1)]

    def layout(cwr_, nr):
        """Gather-order (group -> range -> window -> chunk) layout."""
        n_w_ = cwr_.sum(axis=1)
        cell_start = np.zeros((NW, nr), np.int64)
        slab_meta = []
        c = 0
        for g0, g1 in groups1:
            metas = []
            for r in range(nr):
                s = c
                for w in range(g0, g1):
                    cell_start[w, r] = c
                    c += int(cwr_[w, r])
                metas.append((s, c - s))
            slab_meta.append(metas)
        C_ = c
        gathers = []
        for g in range(len(groups1)):
            for r in range(nr):
                s, n = slab_meta[g][r]
                if n == 0:
                    continue
                # balanced split into ceil(n/SUBCHUNKS) near-equal gathers
                parts = -(-n // SUBCHUNKS)
                bounds_ = [n * j // parts for j in range(parts + 1)]
                for j in range(parts):
                    i, nb = bounds_[j], bounds_[j + 1] - bounds_[j]
                    gathers.append((g, r, i, nb, s + i))
        wc0_ = np.zeros(NW, np.int64)
        wc0_[1:] = np.cumsum(n_w_)[:-1]
        window_chunks = []
        for w in range(NW):
            lst = []
            for r in range(nr):
                for j in range(int(cwr_[w, r])):
                    lst.append((r, int(cell_start[w, r]) + j))
            window_chunks.append(lst)
        return cell_start, slab_meta, gathers, wc0_, window_chunks, C_, n_w_

    cell_start, slab_meta, gathers, wc0, window_chunks, C, n_w = layout(
        cwr, NRANGES
    )

    # phase-2 gathers fetch bf16 (z[2i], z[2i+1]) pair rows (256B) from the
    # halo-exchanged z table; pair ids halve the range count
    NPAIR = (n_nodes + 1) // 2
    NRANGES2 = -(-NPAIR // RSZ)
    rng2_of = (src // 2) // RSZ
    idx2_of = src // 2 - rng2_of * RSZ
    counts2 = np.zeros((n_cores, NW, NRANGES2), np.int64)
    np.add.at(counts2, (owner, wrow, rng2_of), 1)
    cwr2 = (counts2.max(axis=0) + WIN - 1) // WIN
    empty2 = cwr2.sum(axis=1) == 0
    cwr2[empty2, 0] = 1
    cell_start2, slab_meta2, gathers2, wc02, window_chunks2, C2, n_w2 = layout(
        cwr2, NRANGES2
    )

    def wrap16(a_idx, c_):
        # 16-partition engine wrap, replicated to 128 partitions, per the
        # dma_gather idx layout; one column block of 8 per chunk.
        e = np.ascontiguousarray(np.tile(a_idx.reshape(-1, 16).T, (8, 1)))
        assert e.shape == (128, c_ * 8)
        return e

    per_core = []
    for k in range(n_cores):
        m = owner == k
        e_src = src[m]
        e_code_all = code[m]
        e_sw1_all = sw1_all[m]
        e_rng = rng_of[m]
        e_rng2 = rng2_of[m]
        e_wrow = wrow[m]

        # ---- phase 1 layout (sorted by window, src range) ----
        key = e_wrow * NRANGES + e_rng
        order = np.argsort(key, kind="stable")
        s_src = e_src[order]
        s_key = key[order]
        s_code = e_code_all[order]
        s_sw1 = e_sw1_all[order]
        bounds = np.searchsorted(s_key, np.arange(NW * NRANGES + 1))

        # gather-order idx array (pads = 0: they gather row 0 harmlessly and
        # their zero one-hot rows contribute nothing); window-major arrays
        A_idx = np.full(C * WIN, 0, np.int16)
        W_code = np.full(C * WIN, 255.0, np.float32)
        W_sw1 = np.zeros(C * WIN, np.float32)
        for w in range(NW):
            woff = 0
            for r in range(NRANGES):
                a, b = bounds[w * NRANGES + r], bounds[w * NRANGES + r + 1]
                n = b - a
                gbase = int(cell_start[w, r]) * WIN
                wbase = (int(wc0[w]) + woff) * WIN
                woff += int(cwr[w, r])
                if n == 0:
                    continue
                A_idx[gbase : gbase + n] = (s_src[a:b] - r * RSZ).astype(np.int16)
                W_code[wbase : wbase + n] = s_code[a:b]
                W_sw1[wbase : wbase + n] = s_sw1[a:b]

        # ---- phase 2 layout (sorted by window, pair range; parity-split
        # one-hot codes select the z[2i] / z[2i+1] half of each pair row) ----
        e_idx2 = idx2_of[m]
        e_par = (src[m] & 1).astype(np.int64)
        key2 = e_wrow * NRANGES2 + e_rng2
        order2 = np.argsort(key2, kind="stable")
        p_idx = e_idx2[order2]
        p_par = e_par[order2]
        p_key = key2[order2]
        p_code = e_code_all[order2]
        bounds2 = np.searchsorted(p_key, np.arange(NW * NRANGES2 + 1))

        A_idx2 = np.full(C2 * WIN, 0, np.int16)
        W_ev = np.full(C2 * WIN, 255.0, np.float32)
        W_od = np.full(C2 * WIN, 255.0, np.float32)
        for w in range(NW):
            woff = 0
            for r in range(NRANGES2):
                a, b = bounds2[w * NRANGES2 + r], bounds2[w * NRANGES2 + r + 1]
                n = b - a
                gbase = int(cell_start2[w, r]) * WIN
                wbase = (int(wc02[w]) + woff) * WIN
                woff += int(cwr2[w, r])
                if n == 0:
                    continue
                A_idx2[gbase : gbase + n] = p_idx[a:b].astype(np.int16)
                ev = p_par[a:b] == 0
                cseg = p_code[a:b]
                W_ev[wbase : wbase + n] = np.where(ev, cseg, 255.0)
                W_od[wbase : wbase + n] = np.where(ev, 255.0, cseg)

        def tr(a, c_, dt):
            return np.ascontiguousarray(a.reshape(c_, WIN).T.astype(dt))

        def onehot8(codes_flat, c_):
            # host-expanded 0/1 one-hot (exact in fp8), [WIN, c_*WIN]
            codes = codes_flat.reshape(c_, WIN).T  # [WIN, c_]
            oh = codes[:, :, None] == np.arange(WIN, dtype=np.float32)
            return np.ascontiguousarray(
                oh.astype(ml_dtypes.float8_e4m3).reshape(WIN, c_ * WIN)
            )

        # invd in [partition, window] layout for this core's nodes
        nodes = np.arange(P) + k * P
        iv = np.zeros(NW * WIN, np.float32)
        iv[:P] = invd[nodes]
        invd_pw = np.ascontiguousarray(iv.reshape(NW, WIN).T)

        per_core.append(
            dict(eidx=wrap16(A_idx, C), ecode=tr(W_code, C, BF),
                 esw1=tr(W_sw1, C, BF), eidx2=wrap16(A_idx2, C2),
                 eqev=onehot8(W_ev, C2), eqod=onehot8(W_od, C2),
                 invd=invd_pw)
        )

    p = Prep()
    p.P, p.NW, p.rows_last, p.C, p.RSZ = P, NW, rows_last, C, RSZ
    p.nranges = NRANGES
    p.cwr = cwr
    p.n_w = n_w
    p.wc0 = wc0
    p.groups1 = groups1
    p.slab_meta = slab_meta
    p.gathers = gathers
    p.NG = len(gathers)
    p.window_chunks = window_chunks
    p.C2 = C2
    p.nranges2 = NRANGES2
    p.npair = NPAIR
    p.cwr2 = cwr2
    p.n_w2 = n_w2
    p.wc02 = wc02
    p.slab_meta2 = slab_meta2
    p.gathers2 = gathers2
    p.window_chunks2 = window_chunks2
    p.per_core = per_core
    p.n_nodes = n_nodes
    p.n_cores = n_cores
    return p


# ---------------------------------------------------------------------------
# Bass/Tile kernel builder
# ---------------------------------------------------------------------------

def build_gcn(p, F, H, O, gather_bufs=3, gather_bufs2=3):
    NW, C, P, RSZ = p.NW, p.C, p.P, p.RSZ
    NRANGES = p.nranges
    C2, NRANGES2 = p.C2, p.nranges2
    max_nw = int(p.n_w.max())
    max_nw2 = int(p.n_w2.max())
    max_slab = [
        max((p.slab_meta[g][r][1] for g in range(len(p.groups1))), default=0)
        for r in range(NRANGES)
    ]
    max_slab2 = [
        max((p.slab_meta2[g][r][1] for g in range(len(p.groups1))), default=0)
        for r in range(NRANGES2)
    ]
    # gathers grouped by (g, r): list of (chunk_off_in_slab, nb, chunk_start)
    by_slab = {}
    for g, r, i, nb, cs in p.gathers:
        by_slab.setdefault((g, r), []).append((i, nb, cs))
    by_slab2 = {}
    for g, r, i, nb, cs in p.gathers2:
        by_slab2.setdefault((g, r), []).append((i, nb, cs))
    # max window-major chunk columns per group (for the fp8 one-hot tiles)
    max_geq = max(
        int(p.wc02[g1 - 1] + p.n_w2[g1 - 1] - p.wc02[g0])
        for g0, g1 in p.groups1
    )

    nc = bacc.Bacc(
        "TRN2", debug=False, enable_asserts=False, num_devices=p.n_cores,
        num_swdge_queues=NQUEUES,
    )

    x_d = nc.dram_tensor("x", [p.n_nodes, F], BF16, kind="ExternalInput").ap()
    W1_d = nc.dram_tensor("W1", [F, H], BF16, kind="ExternalInput").ap()
    b1_d = nc.dram_tensor("b1", [H, 1], F32, kind="ExternalInput").ap()
    Ws_d = nc.dram_tensor("W_self", [H, O], BF16, kind="ExternalInput").ap()
    Wn_d = nc.dram_tensor("W_neigh", [H, O], BF16, kind="ExternalInput").ap()
    b2_d = nc.dram_tensor("b2", [1, O], BF16, kind="ExternalInput").ap()
    eidx_d = nc.dram_tensor("eidx", [WIN, C * 8], I16, kind="ExternalInput").ap()
    ecode_d = nc.dram_tensor("ecode", [WIN, C], BF16, kind="ExternalInput").ap()
    esw1_d = nc.dram_tensor("esw1", [WIN, C], BF16, kind="ExternalInput").ap()
    eidx2_d = nc.dram_tensor(
        "eidx2", [WIN, C2 * 8], I16, kind="ExternalInput"
    ).ap()
    eqev_d = nc.dram_tensor(
        "eqev", [WIN, C2 * WIN], FP8, kind="ExternalInput"
    ).ap()
    eqod_d = nc.dram_tensor(
        "eqod", [WIN, C2 * WIN], FP8, kind="ExternalInput"
    ).ap()
    invd_d = nc.dram_tensor("invd", [WIN, NW], F32, kind="ExternalInput").ap()
    out_d = nc.dram_tensor("out", [P, O], F32, kind="ExternalOutput").ap()

    qn = [0]

    def next_q():
        q = qn[0]
        qn[0] = (q + 1) % NQUEUES
        return q

    with tile.TileContext(nc, num_cores=p.n_cores) as tc, ExitStack() as ctx:
        const = ctx.enter_context(tc.tile_pool(name="const", bufs=1))
        dram = ctx.enter_context(tc.tile_pool(name="dram", bufs=1, space="DRAM"))

        W1s = const.tile([F, H], BF16)
        nc.sync.dma_start(W1s[:], W1_d)
        Wss = const.tile([H, O], BF16)
        nc.sync.dma_start(Wss[:], Ws_d)
        Wns = const.tile([H, O], BF16)
        nc.sync.dma_start(Wns[:], Wn_d)
        b1s = const.tile([H, 1], F32)
        nc.sync.dma_start(b1s[:], b1_d)
        b2s = const.tile([1, O], BF16)
        nc.sync.dma_start(b2s[:], b2_d)
        invd_s = const.tile([WIN, NW], F32)
        nc.sync.dma_start(invd_s[:], invd_d)

        ones1 = const.tile([1, WIN], BF16)
        nc.vector.memset(ones1[:], 1.0)
        iota = const.tile([WIN, WIN], BF16)
        nc.gpsimd.iota(
            iota[:],
            pattern=[[1, WIN]],
            base=0,
            channel_multiplier=0,
            allow_small_or_imprecise_dtypes=True,
        )

        hT = const.tile([H, NW * WIN], BF16)

        zshard = dram.tile([P, O], BF16)
        # halo-exchanged z, viewed as bf16 (z[2i], z[2i+1]) pair rows so the
        # phase-2 gather descriptors are 256B like phase 1's
        zfull2 = dram.tile([p.npair, 2 * O], BF16, addr_space="Shared")
        # self-term b2 + h @ W_self for every window, filled during the
        # halo exchange so the PE isn't idle while the collective runs
        sb_all = const.tile([WIN, NW * O], BF16)

        def gather_slab(pool, g, r, src_ap, elem, dt, tag, idxs, memset):
            s, n = p.slab_meta[g][r]
            if n == 0:
                return None, s
            t = pool.tile([WIN, max_slab[r], elem], dt, tag=tag)
            if memset:
                nc.vector.memset(t[:], 0.0)
            r0 = r * RSZ
            r1 = min(r0 + RSZ, p.n_nodes)
            for i, nb, cs in by_slab[(g, r)]:
                nc.gpsimd.dma_gather(
                    out_ap=t[:, i : i + nb, :],
                    in_ap=src_ap[r0:r1, :],
                    idxs_ap=idxs[:, cs * 8 : (cs + nb) * 8],
                    num_idxs=nb * WIN,
                    num_idxs_reg=nb * WIN,
                    elem_size=elem,
                    queue_num=next_q(),
                )
            return t, s

        def build_eq(pool, codes, n, c0, mx, tag, weighted=False):
            """Batched one-hot over n chunk columns of `codes`."""
            eq = pool.tile([WIN, mx, WIN], BF16, tag=tag)
            nc.vector.tensor_tensor(
                out=eq[:, :n, :],
                in0=codes[:, c0 : c0 + n].to_broadcast([WIN, n, WIN]),
                in1=iota[:].rearrange("p f -> p () f").to_broadcast([WIN, n, WIN]),
                op=AOT.is_equal,
            )
            if weighted:
                nc.vector.tensor_tensor(
                    out=eq[:, :n, :],
                    in0=eq[:, :n, :],
                    in1=esw1_s[:, c0 : c0 + n].to_broadcast([WIN, n, WIN]),
                    op=AOT.mult,
                )
            return eq

        # ---------------- phase 1 ----------------
        groups1 = p.groups1
        with (
            tc.tile_pool(name="idx1", bufs=1) as idx1p,
            tc.tile_pool(name="xg", bufs=gather_bufs) as xgp,
            tc.tile_pool(name="oh1", bufs=2) as ohp,
            tc.tile_pool(name="aggn", bufs=2 * GROUP1 + 2) as aggp,
            tc.tile_pool(name="zt", bufs=2) as ztp,
            tc.tile_pool(name="psA", bufs=3, space="PSUM") as psA,
            tc.tile_pool(name="psH", bufs=2, space="PSUM") as psH,
            tc.tile_pool(name="psZ", bufs=2, space="PSUM") as psZ,
        ):
            eidx_s = idx1p.tile([WIN, C * 8], I16)
            nc.sync.dma_start(eidx_s[:], eidx_d)
            ecode_s = idx1p.tile([WIN, C], BF16)
            nc.sync.dma_start(ecode_s[:], ecode_d)
            esw1_s = idx1p.tile([WIN, C], BF16)
            nc.sync.dma_start(esw1_s[:], esw1_d)

            pend1 = []  # (w, rows, aggn tile) awaiting tail ops

            def tail1():
                for w, rows, aggn in pend1:
                    wsl = slice(w * WIN, (w + 1) * WIN)
                    hpre = psH.tile([H, WIN], F32, tag="hpre")
                    nc.tensor.matmul(
                        out=hpre[:], lhsT=W1s[:], rhs=aggn[:], start=True,
                        stop=True,
                    )
                    nc.scalar.activation(hT[:, wsl], hpre[:], AFT.Relu, bias=b1s[:])
                for w, rows, aggn in pend1:
                    wsl = slice(w * WIN, (w + 1) * WIN)
                    zp = psZ.tile([WIN, O], F32, tag="zp")
                    nc.tensor.matmul(
                        out=zp[:], lhsT=hT[:, wsl], rhs=Wns[:], start=True,
                        stop=True,
                    )
                    zt = ztp.tile([WIN, O], BF16, tag="zt")
                    nc.vector.tensor_copy(zt[:], zp[:])
                    nc.sync.dma_start(
                        zshard[w * WIN : w * WIN + rows, :], zt[:rows, :]
                    )
                pend1.clear()

            for g, (g0, g1) in enumerate(groups1):
                slabs = {}
                for r in range(NRANGES):
                    t, s = gather_slab(
                        xgp, g, r, x_d, F, BF16, f"xg{r}", eidx_s,
                        memset=g < gather_bufs,
                    )
                    if t is not None:
                        slabs[r] = (t, s)

                tail1()  # tails of group g-1 overlap group g's gathers

                for w in range(g0, g1):
                    rows = p.rows_last if w == NW - 1 else WIN
                    chunks = p.window_chunks[w]

                    eq = build_eq(
                        ohp, ecode_s, int(p.n_w[w]), int(p.wc0[w]), max_nw,
                        "eq", weighted=True,
                    )
                    agg = psA.tile([F, WIN], F32, tag="agg")
                    for jj, (r, gid) in enumerate(chunks):
                        t, s = slabs[r]
                        nc.tensor.matmul(
                            out=agg[:],
                            lhsT=t[:, gid - s, :],
                            rhs=eq[:, jj, :],
                            start=(jj == 0),
                            stop=(jj == len(chunks) - 1),
                        )

                    aggn = aggp.tile([F, WIN], BF16, tag="aggn")
                    nc.scalar.activation(aggn[:], agg[:], AFT.Copy)
                    pend1.append((w, rows, aggn))

            tail1()

        # ---------------- halo exchange ----------------
        nc.gpsimd.collective_compute(
            "AllGather",
            AOT.bypass,
            replica_groups=[list(range(p.n_cores))],
            ins=[zshard.opt()],
            outs=[zfull2.opt()],
        )

        # overlap the collective with the self-term matmuls (independent of z)
        with tc.tile_pool(name="psB", bufs=3, space="PSUM") as psB:
            for w in range(NW):
                wsl = slice(w * WIN, (w + 1) * WIN)
                sb = psB.tile([WIN, O], F32, tag="sb")
                nc.tensor.matmul(
                    out=sb[:], lhsT=ones1[:], rhs=b2s[:], start=True, stop=False
                )
                nc.tensor.matmul(
                    out=sb[:], lhsT=hT[:, wsl], rhs=Wss[:], start=False, stop=True
                )
                nc.scalar.activation(sb_all[:, w * O : (w + 1) * O], sb[:], AFT.Copy)

        # ---------------- phase 2 ----------------
        with (
            tc.tile_pool(name="idx2", bufs=1) as idx2p,
            tc.tile_pool(name="zg", bufs=gather_bufs2) as zgp,
            tc.tile_pool(name="oh2", bufs=2) as ohp2,
            tc.tile_pool(name="nm", bufs=2) as nmp,
            tc.tile_pool(name="nmc", bufs=2 * GROUP1 + 2) as nmcp,
            tc.tile_pool(name="ot", bufs=2) as otp,
            tc.tile_pool(name="psN", bufs=3, space="PSUM") as psN,
        ):
            eidx2_s = idx2p.tile([WIN, C2 * 8], I16)
            nc.sync.dma_start(eidx2_s[:], eidx2_d)

            pend2 = []  # (w, rows, nm SBUF copy)

            def tail2():
                for w, rows, nm in pend2:
                    # nm * invd[dst] (partition-wise) then + (h@Ws + b2)
                    nms = nmp.tile([WIN, O], F32, tag="nms")
                    nc.vector.tensor_scalar(
                        out=nms[:], in0=nm[:], scalar1=invd_s[:, w : w + 1],
                        scalar2=None, op0=AOT.mult,
                    )
                    outt = otp.tile([WIN, O], F32, tag="outt")
                    nc.vector.tensor_tensor(
                        outt[:], nms[:], sb_all[:, w * O : (w + 1) * O],
                        op=AOT.add,
                    )
                    nc.sync.dma_start(
                        out_d[w * WIN : w * WIN + rows, :], outt[:rows, :]
                    )
                pend2.clear()

            for g, (g0, g1) in enumerate(groups1):
                slabs = {}
                for r in range(NRANGES2):
                    s, n = p.slab_meta2[g][r]
                    if n == 0:
                        continue
                    t = zgp.tile([WIN, max_slab2[r], 2 * O], BF16, tag=f"zg{r}")
                    if g < gather_bufs2:
                        nc.vector.memset(t[:], 0.0)
                    lo = r * RSZ
                    hi = min(lo + RSZ, p.npair)
                    for i, nb, cs in by_slab2[(g, r)]:
                        nc.gpsimd.dma_gather(
                            out_ap=t[:, i : i + nb, :],
                            in_ap=zfull2.opt()[lo:hi, :],
                            idxs_ap=eidx2_s[:, cs * 8 : (cs + nb) * 8],
                            num_idxs=nb * WIN,
                            num_idxs_reg=nb * WIN,
                            elem_size=2 * O,
                            queue_num=next_q(),
                        )
                    slabs[r] = (t, s)

                # host-precomputed fp8 parity one-hots for this group's
                # window-major chunk columns (replaces DVE is_equal builds)
                q0 = int(p.wc02[g0])
                q1 = int(p.wc02[g1 - 1] + p.n_w2[g1 - 1])
                eqe = ohp2.tile([WIN, max_geq, WIN], FP8, tag="eqe")
                nc.sync.dma_start(
                    eqe[:, 0 : q1 - q0, :], eqev_d[:, q0 * WIN : q1 * WIN]
                )
                eqo = ohp2.tile([WIN, max_geq, WIN], FP8, tag="eqo")
                nc.sync.dma_start(
                    eqo[:, 0 : q1 - q0, :], eqod_d[:, q0 * WIN : q1 * WIN]
                )

                tail2()

                for w in range(g0, g1):
                    rows = p.rows_last if w == NW - 1 else WIN
                    chunks = p.window_chunks2[w]
                    c20 = int(p.wc02[w])

                    nm = psN.tile([WIN, O], F32, tag="nm")
                    for jj, (r, gid) in enumerate(chunks):
                        t, s = slabs[r]
                        jq = c20 - q0 + jj
                        nc.tensor.matmul(
                            out=nm[:],
                            lhsT=eqe[:, jq, :],
                            rhs=t[:, gid - s, 0:O],
                            start=(jj == 0),
                            stop=False,
                        )
                        nc.tensor.matmul(
                            out=nm[:],
                            lhsT=eqo[:, jq, :],
                            rhs=t[:, gid - s, O : 2 * O],
                            start=False,
                            stop=(jj == len(chunks) - 1),
                        )
                    nmc = nmcp.tile([WIN, O], F32, tag="nmc")
                    nc.scalar.activation(nmc[:], nm[:], AFT.Copy)
                    pend2.append((w, rows, nmc))

            tail2()

    nc.compile()
    return nc


# ---------------------------------------------------------------------------
# Entry point
# ---------------------------------------------------------------------------

_CACHE = {}


def _get_compiled(p, F, H, O):
    key = (p.n_nodes, p.n_cores, p.C, tuple(map(tuple, p.cwr.tolist())), F, H, O)
    if key not in _CACHE:
        import time as _time

        t0 = _time.time()
        _CACHE[key] = build_gcn(p, F, H, O)
        if os.environ.get("GCN_VERBOSE"):
            print(f"[gcn] build+bass-compile: {_time.time() - t0:.1f}s", flush=True)
    return _CACHE[key]


def make_in_maps(p, inputs):
    H = np.asarray(inputs["W1"]).shape[1]
    O = np.asarray(inputs["W_self"]).shape[1]
    base = dict(
        x=np.ascontiguousarray(np.asarray(inputs["x"], np.float32).astype(BF)),
        W1=np.ascontiguousarray(np.asarray(inputs["W1"], np.float32).astype(BF)),
        b1=np.ascontiguousarray(np.asarray(inputs["b1"], np.float32).reshape(H, 1)),
        W_self=np.ascontiguousarray(
            np.asarray(inputs["W_self"], np.float32).astype(BF)
        ),
        W_neigh=np.ascontiguousarray(
            np.asarray(inputs["W_neigh"], np.float32).astype(BF)
        ),
        b2=np.ascontiguousarray(
            np.asarray(inputs["b2"], np.float32).reshape(1, O).astype(BF)
        ),
    )
    in_maps = []
    for k in range(p.n_cores):
        m = dict(base)
        m.update(p.per_core[k])
        in_maps.append(m)
    return in_maps


def kernel(**inputs):
    x = np.asarray(inputs["x"])
    src = np.asarray(inputs["src"])
    dst = np.asarray(inputs["dst"])
    n_nodes, F = x.shape
    H = np.asarray(inputs["W1"]).shape[1]
    O = np.asarray(inputs["W_self"]).shape[1]

    p = prepare(src, dst, n_nodes)
    nc = _get_compiled(p, F, H, O)
    in_maps = make_in_maps(p, inputs)
    res = run_bass_kernel_spmd(
        nc, in_maps, core_ids=list(range(p.n_cores)),
        trace=bool(int(os.environ.get("GCN_TRACE", "0"))),
    )
    if os.environ.get("GCN_RESULT_HOOK"):
        _CACHE["last_results"] = res
    out = np.concatenate([r["out"] for r in res.results], axis=0)
    return out.astype(np.float32)
